# revision 4
# baseline (speedup 1.0000x reference)
"""GatedMultiScaleRetention on 8 TRN2 NeuronCores (Bass/Tile).

Sharding: head-parallel (2 heads/core) for qkv+retention+groupnorm, then one
AllToAll to switch to sequence-parallel (256 seq cols/core) for the gate/out
projections. Host assembles the per-core output column shards.

Retention is computed in chunked form (blocks of 128 positions): within-block
decay folded into exp-scaled q/k, cross-block via the recurrent state
S <- exp(c_last) * (S + k~^T v), which is algebraically identical to the
reference's dense 2048x2048 Delta-masked attention.
"""

import numpy as np

import concourse.bass as bass
import concourse.tile as tile
from concourse import bacc, mybir
from concourse import bass_utils

S = 2048          # sequence length
D = 2048          # d_model
H = 16            # heads
DH = 128          # head dim
NCORES = 8
HL = H // NCORES  # heads per core = 2
T = S // 128      # 16 seq tiles of 128
CL = HL * DH      # local channels = 256
EPS = 1e-5

f32 = mybir.dt.float32
f32r = mybir.dt.float32r
Act = mybir.ActivationFunctionType
Alu = mybir.AluOpType
RG = [list(range(NCORES))]

_compiled = None


def _emit(nc, tc, io):
    ten = nc.tensor
    vec = nc.vector
    act = nc.scalar

    xT_re = io["xT"].ap().rearrange("(ct p) s -> p ct s", p=128)        # [128,16,2048]
    wqkv_re = io["wqkv"].ap().rearrange("(ct p) r -> p ct r", p=128)    # [128,16,770]
    trig_res = {
        k: io[k].ap().rearrange("(t p) j -> p t j", p=128) for k in
        ("cosq", "sinq", "cosk", "sink")
    }

    with (
        tc.tile_pool(name="dpool", bufs=1, space="DRAM") as dpool,
        tc.tile_pool(name="consts", bufs=1) as consts,
    ):
        bounce = dpool.tile([S, CL], f32, name="bounce")
        recv = dpool.tile([S, CL], f32, name="recv")

        # ---- constants ----
        triu_sb = consts.tile([128, 128], f32, name="triu_sb")
        nc.sync.dma_start(out=triu_sb, in_=io["triu"].ap())
        slow_sb = consts.tile([128, 128], f32, name="slow_sb")
        nc.sync.dma_start(out=slow_sb, in_=io["slow"].ap())
        ident_sb = consts.tile([128, 128], f32, name="ident_sb")
        nc.sync.dma_start(out=ident_sb, in_=io["ident"].ap())
        balbc_sb = consts.tile([128, HL], f32, name="balbc_sb")
        nc.sync.dma_start(out=balbc_sb, in_=io["balbc"].ap())
        abbc_sb = consts.tile([128, HL], f32, name="abbc_sb")
        nc.sync.dma_start(out=abbc_sb, in_=io["abbc"].ap())
        gnw_sb = consts.tile([DH, HL], f32, name="gnw_sb")
        nc.sync.dma_start(out=gnw_sb, in_=io["gnw"].ap())
        gnb_sb = consts.tile([DH, HL], f32, name="gnb_sb")
        nc.sync.dma_start(out=gnb_sb, in_=io["gnb"].ap())
        eps_sb = consts.tile([128, 1], f32, name="eps_sb")
        vec.memset(eps_sb, EPS)
        trig_sb = {}
        for k, re_ap in trig_res.items():
            trig_sb[k] = consts.tile([128, T, 64], f32, name=f"{k}_sb")
            nc.sync.dma_start(out=trig_sb[k], in_=re_ap)

        # ---- phases A (qkv) + B (retention + groupnorm) ----
        with (
            tc.tile_pool(name="wqkvp", bufs=1) as wqkvp,
            tc.tile_pool(name="actsp", bufs=1) as actsp,
            tc.tile_pool(name="xp", bufs=2) as xp,
            tc.tile_pool(name="sp", bufs=2) as sp,
            tc.tile_pool(name="bp", bufs=2) as bp,
            tc.tile_pool(name="psA", bufs=1, space="PSUM") as psA,
            tc.tile_pool(name="psB", bufs=1, space="PSUM") as psB,
        ):
            wsb = wqkvp.tile([128, T, 770], f32r, name="wsb")
            nc.sync.dma_start(out=wsb, in_=wqkv_re)

            qts = [actsp.tile([128, CL], f32, name=f"q{t}") for t in range(T)]
            kts = [actsp.tile([128, CL], f32, name=f"k{t}") for t in range(T)]
            vts = [actsp.tile([128, CL], f32, name=f"v{t}") for t in range(T)]
            retn = [actsp.tile([128, S], f32, name=f"retn{h}") for h in range(HL)]
            S_sb = actsp.tile([128, HL, DH], f32, name="S_sb")
            for h in range(HL):
                nc.sync.dma_start(out=S_sb[:, h, :], in_=io["state"].ap()[h])

            for t in range(T):
                # --- A: qkv + alpha projection for seq tile t ---
                xt = xp.tile([128, T, 128], f32r, name="xt")
                nc.sync.dma_start(out=xt, in_=xT_re[:, :, t * 128:(t + 1) * 128])
                pqv = psA.tile([128, 770], f32, name="pqv")
                for ci in range(T):
                    ten.matmul(pqv[:, 0:512], xt[:, ci, :], wsb[:, ci, 0:512],
                               start=(ci == 0), stop=(ci == T - 1))
                for ci in range(T):
                    ten.matmul(pqv[:, 512:770], xt[:, ci, :], wsb[:, ci, 512:770],
                               start=(ci == 0), stop=(ci == T - 1))
                act.copy(qts[t][:], pqv[:, 0:256])
                act.copy(kts[t][:], pqv[:, 256:512])
                act.copy(vts[t][:], pqv[:, 512:768])

                # --- alpha chain ---
                sig = sp.tile([128, HL], f32, name="sig")
                for h in range(HL):
                    act.activation(sig[:, h:h + 1], pqv[:, 768 + h:769 + h],
                                   Act.Sigmoid, bias=balbc_sb[:, h:h + 1])
                alf = sp.tile([128, HL], f32, name="alf")
                for h in range(HL):
                    vec.tensor_scalar(out=alf[:, h:h + 1], in0=sig[:, h:h + 1],
                                      scalar1=abbc_sb[:, h:h + 1], scalar2=8.0,
                                      op0=Alu.mult, op1=Alu.mult)
                ee = sp.tile([128, HL], f32, name="ee")
                act.activation(ee[:], alf[:], Act.Exp)
                kd = sp.tile([128, HL], f32, name="kd")
                vec.tensor_scalar(out=kd[:], in0=ee[:], scalar1=-1.0, scalar2=1.0,
                                  op0=Alu.mult, op1=Alu.add)
                pc = psA.tile([128, 4], f32, name="pc")
                ten.matmul(pc[:, 0:2], triu_sb[:], alf[:], start=True, stop=True)
                ten.matmul(pc[:, 2:4], slow_sb[:], alf[:], start=True, stop=True)
                expc = sp.tile([128, HL], f32, name="expc")
                act.activation(expc[:], pc[:, 0:2], Act.Exp)
                expnc = sp.tile([128, HL], f32, name="expnc")
                act.activation(expnc[:], pc[:, 0:2], Act.Exp, scale=-1.0)
                expcr = sp.tile([128, HL], f32, name="expcr")
                act.activation(expcr[:], pc[:, 2:4], Act.Exp)
                ksc = sp.tile([128, HL], f32, name="ksc")
                vec.tensor_mul(ksc[:], kd[:], expnc[:])
                dec = sp.tile([128, HL], f32, name="dec")
                vec.tensor_mul(dec[:], expc[:], expcr[:])

                cq = trig_sb["cosq"][:, t, :]
                sq = trig_sb["sinq"][:, t, :]
                ck = trig_sb["cosk"][:, t, :]
                sk = trig_sb["sink"][:, t, :]

                for h in range(HL):
                    hs = slice(h * DH, (h + 1) * DH)
                    # q~ = xpos(q) * sc * exp(c)
                    qsc = bp.tile([128, 128], f32, name="qsc")
                    vec.tensor_scalar_mul(qsc[:], qts[t][:, hs], expc[:, h:h + 1])
                    qrot = bp.tile([128, 128], f32, name="qrot")
                    vec.tensor_mul(qrot[:, 0:64], qsc[:, 0:64], cq)
                    vec.tensor_mul(qrot[:, 64:128], qsc[:, 64:128], cq)
                    qcr = bp.tile([128, 128], f32, name="qcr")
                    vec.tensor_mul(qcr[:, 0:64], qsc[:, 64:128], sq)
                    vec.tensor_mul(qcr[:, 64:128], qsc[:, 0:64], sq)
                    vec.tensor_sub(qrot[:, 0:64], qrot[:, 0:64], qcr[:, 0:64])
                    vec.tensor_add(qrot[:, 64:128], qrot[:, 64:128], qcr[:, 64:128])
                    # k~ = xpos_inv(k) * (1-e^a) * exp(-c)
                    ksl = bp.tile([128, 128], f32, name="ksl")
                    vec.tensor_scalar_mul(ksl[:], kts[t][:, hs], ksc[:, h:h + 1])
                    krot = bp.tile([128, 128], f32, name="krot")
                    vec.tensor_mul(krot[:, 0:64], ksl[:, 0:64], ck)
                    vec.tensor_mul(krot[:, 64:128], ksl[:, 64:128], ck)
                    kcr = bp.tile([128, 128], f32, name="kcr")
                    vec.tensor_mul(kcr[:, 0:64], ksl[:, 64:128], sk)
                    vec.tensor_mul(kcr[:, 64:128], ksl[:, 0:64], sk)
                    vec.tensor_sub(krot[:, 0:64], krot[:, 0:64], kcr[:, 0:64])
                    vec.tensor_add(krot[:, 64:128], krot[:, 64:128], kcr[:, 64:128])

                    # transposes to [d, s]
                    pqT = psB.tile([128, 128], f32, name="pqT")
                    ten.transpose(pqT[:], qrot[:], ident_sb[:])
                    qT = bp.tile([128, 128], f32, name="qT")
                    act.copy(qT[:], pqT[:])
                    pkT = psB.tile([128, 128], f32, name="pkT")
                    ten.transpose(pkT[:], krot[:], ident_sb[:])
                    kT = bp.tile([128, 128], f32, name="kT")
                    act.copy(kT[:], pkT[:])

                    # attn^T [j, i], causal-masked
                    pa = psB.tile([128, 128], f32, name="pa")
                    ten.matmul(pa[:], kT[:], qT[:], start=True, stop=True)
                    am = bp.tile([128, 128], f32, name="am")
                    vec.tensor_mul(am[:], pa[:], triu_sb[:])

                    # ret[i, e] = attn @ v + q~ @ S
                    pr = psB.tile([128, 128], f32, name="pr")
                    ten.matmul(pr[:], am[:], vts[t][:, hs], start=True, stop=False)
                    ten.matmul(pr[:], qT[:], S_sb[:, h, :], start=False, stop=True)

                    # state update: S = dec * (S + k~^T v)
                    pkv = psB.tile([128, 128], f32, name="pkv")
                    ten.matmul(pkv[:], krot[:], vts[t][:, hs], start=True, stop=True)
                    Stmp = bp.tile([128, 128], f32, name="Stmp")
                    vec.tensor_add(Stmp[:], S_sb[:, h, :], pkv[:])
                    vec.tensor_scalar_mul(S_sb[:, h, :], Stmp[:], dec[:, h:h + 1])

                    # groupnorm over e (free axis), then affine+transpose
                    st6 = bp.tile([128, 6], f32, name="st6")
                    vec.bn_stats(st6[:], pr[:])
                    mv = bp.tile([128, 2], f32, name="mv")
                    vec.bn_aggr(mv[:], st6[:])
                    sdv = bp.tile([128, 1], f32, name="sdv")
                    act.activation(sdv[:], mv[:, 1:2], Act.Sqrt, bias=eps_sb[:, 0:1])
                    rst = bp.tile([128, 1], f32, name="rst")
                    vec.reciprocal(rst[:], sdv[:])
                    xn = bp.tile([128, 128], f32, name="xn")
                    vec.tensor_scalar(out=xn[:], in0=pr[:], scalar1=mv[:, 0:1],
                                      scalar2=rst[:, 0:1], op0=Alu.subtract,
                                      op1=Alu.mult)
                    pt = psB.tile([128, 128], f32, name="pa")
                    ten.transpose(pt[:], xn[:], ident_sb[:])
                    vec.tensor_scalar(out=retn[h][:, t * 128:(t + 1) * 128],
                                      in0=pt[:], scalar1=gnw_sb[:, h:h + 1],
                                      scalar2=gnb_sb[:, h:h + 1], op0=Alu.mult,
                                      op1=Alu.add)

            # state out
            for h in range(HL):
                nc.sync.dma_start(out=io["new_state"].ap()[h], in_=S_sb[:, h, :])

            # pack bounce: bounce[j*256 + h*128 + e, sw] = retn[h][e, j*256+sw]
            for j in range(NCORES):
                for h in range(HL):
                    nc.sync.dma_start(
                        out=bounce[j * CL + h * DH: j * CL + (h + 1) * DH, :],
                        in_=retn[h][:, j * CL:(j + 1) * CL])

        nc.gpsimd.collective_compute(
            "AllToAll", Alu.bypass, replica_groups=RG,
            ins=[bounce.opt()], outs=[recv.opt()],
        )

        # ---- phases C (gate) + D (out), sequence-parallel ----
        with (
            tc.tile_pool(name="rtp", bufs=1) as rtp,
            tc.tile_pool(name="wp", bufs=3) as wp,
            tc.tile_pool(name="cp", bufs=2) as cp,
            tc.tile_pool(name="psD", bufs=1, space="PSUM") as psD,
        ):
            rtr = [rtp.tile([128, CL], f32r, name=f"rtr{ci}") for ci in range(T)]
            rt32 = [rtp.tile([128, CL], f32, name=f"rt32_{ci}") for ci in range(T)]
            zr = [rtp.tile([128, CL], f32r, name=f"zr{ci}") for ci in range(T)]
            for ci in range(T):
                nc.gpsimd.dma_start(out=rtr[ci][:], in_=recv[ci * 128:(ci + 1) * 128, :])
                nc.sync.dma_start(out=rt32[ci][:], in_=recv[ci * 128:(ci + 1) * 128, :])

            wg_ap = io["wg"].ap()
            wo_ap = io["wo"].ap()
            for og in range(2):
                pg = [psD.tile([128, CL], f32, name=f"pg{oi}") for oi in range(8)]
                for ci in range(T):
                    wgt = wp.tile([128, 1024], f32r, name="wgt")
                    nc.sync.dma_start(
                        out=wgt, in_=wg_ap[ci * 128:(ci + 1) * 128,
                                          og * 1024:(og + 1) * 1024])
                    for oi in range(8):
                        ten.matmul(pg[oi][:], wgt[:, oi * 128:(oi + 1) * 128],
                                   rtr[ci][:], start=(ci == 0), stop=(ci == T - 1))
                for oi in range(8):
                    o = og * 8 + oi
                    sil = cp.tile([128, CL], f32, name="sil")
                    act.activation(sil[:], pg[oi][:], Act.Silu)
                    z32 = cp.tile([128, CL], f32, name="z32")
                    vec.tensor_mul(z32[:], sil[:], rt32[o][:])
                    nc.gpsimd.dma_start(out=zr[o][:], in_=z32[:])

            for og in range(2):
                po = [psD.tile([128, CL], f32, name=f"pg{oi}") for oi in range(8)]
                for ci in range(T):
                    wot = wp.tile([128, 1024], f32r, name="wot")
                    nc.sync.dma_start(
                        out=wot, in_=wo_ap[ci * 128:(ci + 1) * 128,
                                           og * 1024:(og + 1) * 1024])
                    for oi in range(8):
                        ten.matmul(po[oi][:], wot[:, oi * 128:(oi + 1) * 128],
                                   zr[ci][:], start=(ci == 0), stop=(ci == T - 1))
                for oi in range(8):
                    o = og * 8 + oi
                    ot = cp.tile([128, CL], f32, name="ot")
                    act.copy(ot[:], po[oi][:])
                    nc.sync.dma_start(out=io["outT"].ap()[o * 128:(o + 1) * 128, :],
                                      in_=ot[:])


def _build():
    nc = bacc.Bacc("TRN2", target_bir_lowering=False, debug=False,
                   num_devices=NCORES)
    io = {}
    io["xT"] = nc.dram_tensor("xT", [D, S], f32r, kind="ExternalInput")
    io["wqkv"] = nc.dram_tensor("wqkv", [D, 770], f32r, kind="ExternalInput")
    io["wg"] = nc.dram_tensor("wg", [D, D], f32r, kind="ExternalInput")
    io["wo"] = nc.dram_tensor("wo", [D, D], f32r, kind="ExternalInput")
    io["state"] = nc.dram_tensor("state", [HL, DH, DH], f32, kind="ExternalInput")
    io["balbc"] = nc.dram_tensor("balbc", [128, HL], f32, kind="ExternalInput")
    io["abbc"] = nc.dram_tensor("abbc", [128, HL], f32, kind="ExternalInput")
    io["gnw"] = nc.dram_tensor("gnw", [DH, HL], f32, kind="ExternalInput")
    io["gnb"] = nc.dram_tensor("gnb", [DH, HL], f32, kind="ExternalInput")
    for k in ("cosq", "sinq", "cosk", "sink"):
        io[k] = nc.dram_tensor(k, [S, 64], f32, kind="ExternalInput")
    io["triu"] = nc.dram_tensor("triu", [128, 128], f32, kind="ExternalInput")
    io["slow"] = nc.dram_tensor("slow", [128, 128], f32, kind="ExternalInput")
    io["ident"] = nc.dram_tensor("ident", [128, 128], f32, kind="ExternalInput")
    io["outT"] = nc.dram_tensor("outT", [D, CL], f32, kind="ExternalOutput")
    io["new_state"] = nc.dram_tensor("new_state", [HL, DH, DH], f32,
                                     kind="ExternalOutput")

    with tile.TileContext(nc) as tc:
        _emit(nc, tc, io)
    nc.compile()
    return nc


def _prep_inputs(x, state, W_qkv, W_alpha, b_alpha, alpha_base, gn_weight,
                 gn_bias, W_out, W_gate, offset):
    asf = lambda a: np.ascontiguousarray(np.asarray(a, dtype=np.float32))
    x = asf(x); state = asf(state); W_qkv = asf(W_qkv); W_alpha = asf(W_alpha)
    b_alpha = asf(b_alpha); alpha_base = asf(alpha_base)
    gn_weight = asf(gn_weight); gn_bias = asf(gn_bias)
    W_out = asf(W_out); W_gate = asf(W_gate)
    off = float(np.asarray(offset))

    xT = np.ascontiguousarray(x.T)
    wgT = np.ascontiguousarray(W_gate.T)
    woT = np.ascontiguousarray(W_out.T)

    half = 64
    freq = (1.0 / (10000.0 ** (np.arange(half, dtype=np.float32) / half)))
    pos = np.arange(S, dtype=np.float32) + np.float32(off)
    ang = pos[:, None] * freq[None, :]
    zeta = ((np.arange(half, dtype=np.float32) * 2.0 + 0.4 * DH) / (1.4 * DH))
    sc = zeta[None, :] ** (pos[:, None] / 512.0)
    cosq = asf(np.cos(ang) * sc)
    sinq = asf(np.sin(ang) * sc)
    cosk = asf(np.cos(ang) / sc)
    sink = asf(np.sin(ang) / sc)
    triu = asf(np.triu(np.ones((128, 128), dtype=np.float32)))
    slow = asf(np.tril(np.ones((128, 128), dtype=np.float32), -1))
    ident = asf(np.eye(128, dtype=np.float32))

    in_maps = []
    for r in range(NCORES):
        h0 = HL * r
        wq = W_qkv[h0 * DH:(h0 + HL) * DH]
        wk = W_qkv[D + h0 * DH: D + (h0 + HL) * DH]
        wv = W_qkv[2 * D + h0 * DH: 2 * D + (h0 + HL) * DH]
        wa = W_alpha[h0:h0 + HL]
        wqkvT = np.ascontiguousarray(np.concatenate([wq, wk, wv, wa], axis=0).T)
        in_maps.append({
            "xT": xT,
            "wqkv": wqkvT,
            "wg": wgT,
            "wo": woT,
            "state": np.ascontiguousarray(state[h0:h0 + HL]),
            "balbc": np.ascontiguousarray(np.tile(b_alpha[h0:h0 + HL][None, :], (128, 1))),
            "abbc": np.ascontiguousarray(np.tile(alpha_base[h0:h0 + HL][None, :], (128, 1))),
            "gnw": np.ascontiguousarray(gn_weight[h0 * DH:(h0 + HL) * DH].reshape(HL, DH).T),
            "gnb": np.ascontiguousarray(gn_bias[h0 * DH:(h0 + HL) * DH].reshape(HL, DH).T),
            "cosq": cosq, "sinq": sinq, "cosk": cosk, "sink": sink,
            "triu": triu, "slow": slow, "ident": ident,
        })
    return in_maps


def kernel(x, state, W_qkv, W_alpha, b_alpha, alpha_base, gn_weight, gn_bias,
           W_out, W_gate, offset, _trace=False):
    global _compiled
    if _compiled is None:
        _compiled = _build()
    nc = _compiled
    in_maps = _prep_inputs(x, state, W_qkv, W_alpha, b_alpha, alpha_base,
                           gn_weight, gn_bias, W_out, W_gate, offset)
    res = bass_utils.run_bass_kernel_spmd(nc, in_maps,
                                          core_ids=list(range(NCORES)),
                                          trace=_trace)
    kernel.last_results = res
    out = np.concatenate([res.results[r]["outT"] for r in range(NCORES)],
                         axis=1).T.copy()
    new_state = np.concatenate([res.results[r]["new_state"]
                                for r in range(NCORES)], axis=0)
    return out, new_state


# revision 9
# speedup vs baseline: 1.2309x; 1.2309x over previous
"""GatedMultiScaleRetention on 8 TRN2 NeuronCores (Bass/Tile).

Sharding: head-parallel (2 heads/core) for qkv+retention+groupnorm, then one
AllToAll to switch to sequence-parallel (256 seq cols/core) for the gate/out
projections. Host assembles the per-core output column shards.

Retention is computed in chunked form (blocks of 128 positions): within-block
decay folded into exp-scaled q/k, cross-block via the recurrent state
S <- exp(c_last) * (S + k~^T v), which is algebraically identical to the
reference's dense 2048x2048 Delta-masked attention.
"""

import numpy as np
import ml_dtypes

import concourse.bass as bass
import concourse.tile as tile
from concourse import bacc, mybir
from concourse import bass_utils

S = 2048          # sequence length
D = 2048          # d_model
H = 16            # heads
DH = 128          # head dim
NCORES = 8
HL = H // NCORES  # heads per core = 2
T = S // 128      # 16 seq tiles of 128
CL = HL * DH      # local channels = 256
EPS = 1e-5

f32 = mybir.dt.float32
f32r = mybir.dt.float32r
bf16 = mybir.dt.bfloat16
Act = mybir.ActivationFunctionType
Alu = mybir.AluOpType
RG = [list(range(NCORES))]

_compiled = None


def _emit(nc, tc, io):
    ten = nc.tensor
    vec = nc.vector
    act = nc.scalar

    xT_re = io["xT"].ap().rearrange("(ct p) s -> p ct s", p=128)        # [128,16,2048]
    wqkv_re = io["wqkv"].ap().rearrange("(ct p) r -> p ct r", p=128)    # [128,16,770]
    trig_res = {
        k: io[k].ap().rearrange("(t p) j -> p t j", p=128) for k in
        ("cosq", "sinq", "cosk", "sink")
    }

    with (
        tc.tile_pool(name="dpool", bufs=1, space="DRAM") as dpool,
        tc.tile_pool(name="consts", bufs=1) as consts,
    ):
        bounce = dpool.tile([S, CL], f32, name="bounce")
        recv = dpool.tile([S, CL], f32, name="recv")

        # ---- constants ----
        triu_sb = consts.tile([128, 128], f32, name="triu_sb")
        nc.sync.dma_start(out=triu_sb, in_=io["triu"].ap())
        slow_sb = consts.tile([128, 128], f32, name="slow_sb")
        nc.sync.dma_start(out=slow_sb, in_=io["slow"].ap())
        ident_sb = consts.tile([128, 128], f32, name="ident_sb")
        nc.sync.dma_start(out=ident_sb, in_=io["ident"].ap())
        balbc_sb = consts.tile([128, HL], f32, name="balbc_sb")
        nc.sync.dma_start(out=balbc_sb, in_=io["balbc"].ap())
        balneg_sb = consts.tile([128, HL], f32, name="balneg_sb")
        vec.tensor_scalar_mul(balneg_sb[:], balbc_sb[:], -1.0)
        abbc_sb = consts.tile([128, HL], f32, name="abbc_sb")
        nc.sync.dma_start(out=abbc_sb, in_=io["abbc"].ap())
        gnw_sb = consts.tile([DH, HL], f32, name="gnw_sb")
        nc.sync.dma_start(out=gnw_sb, in_=io["gnw"].ap())
        gnb_sb = consts.tile([DH, HL], f32, name="gnb_sb")
        nc.sync.dma_start(out=gnb_sb, in_=io["gnb"].ap())
        eps_sb = consts.tile([128, 1], f32, name="eps_sb")
        vec.memset(eps_sb, EPS)
        trig_sb = {}
        for k, re_ap in trig_res.items():
            trig_sb[k] = consts.tile([128, T, 64], f32, name=f"{k}_sb")
            nc.sync.dma_start(out=trig_sb[k], in_=re_ap)

        # ---- phases A (qkv) + B (retention + groupnorm) ----
        with (
            tc.tile_pool(name="wqkvp", bufs=1) as wqkvp,
            tc.tile_pool(name="actsp", bufs=1) as actsp,
            tc.tile_pool(name="xp", bufs=3) as xp,
            tc.tile_pool(name="sp", bufs=2) as sp,
            tc.tile_pool(name="bp", bufs=2) as bp,
            tc.tile_pool(name="psA", bufs=2, space="PSUM") as psA,
            tc.tile_pool(name="psB", bufs=1, space="PSUM") as psB,
        ):
            wsb = wqkvp.tile([128, T, 770], f32r, name="wsb")
            nc.sync.dma_start(out=wsb, in_=wqkv_re)

            vts = [actsp.tile([128, CL], f32, name=f"v{t}") for t in range(T)]
            retn = [actsp.tile([128, S], f32, name=f"retn{h}") for h in range(HL)]
            rwall = actsp.tile([128, T * HL, 128], f32, name="rwall")
            mvall = actsp.tile([128, 2 * T * HL], f32, name="mvall")
            rsall = actsp.tile([128, T * HL], f32, name="rsall")
            S_sb = actsp.tile([128, HL, DH], f32, name="S_sb")
            for h in range(HL):
                nc.sync.dma_start(out=S_sb[:, h, :], in_=io["state"].ap()[h])

            for t in range(T):
                # --- A: qkv + alpha projection for seq tile t ---
                xt = xp.tile([128, T, 128], f32r, name="xt")
                nc.sync.dma_start(out=xt, in_=xT_re[:, :, t * 128:(t + 1) * 128])
                pqv = psA.tile([128, 770], f32, name="pqv")
                for ci in range(T):
                    ten.matmul(pqv[:, 0:512], xt[:, ci, :], wsb[:, ci, 0:512],
                               start=(ci == 0), stop=(ci == T - 1))
                for ci in range(T):
                    ten.matmul(pqv[:, 512:770], xt[:, ci, :], wsb[:, ci, 512:770],
                               start=(ci == 0), stop=(ci == T - 1))
                vec.tensor_copy(vts[t][:], pqv[:, 512:768])

                # --- alpha chain (ACT only ever runs Exp here) ---
                eneg = sp.tile([128, HL], f32, name="eneg")
                for h in range(HL):
                    act.activation(eneg[:, h:h + 1], pqv[:, 768 + h:769 + h],
                                   Act.Exp, bias=balneg_sb[:, h:h + 1], scale=-1.0)
                sig = sp.tile([128, HL], f32, name="sig")
                vec.tensor_scalar_add(sig[:], eneg[:], 1.0)
                vec.reciprocal(sig[:], sig[:])
                alf = sp.tile([128, HL], f32, name="alf")
                for h in range(HL):
                    vec.tensor_scalar(out=alf[:, h:h + 1], in0=sig[:, h:h + 1],
                                      scalar1=abbc_sb[:, h:h + 1], scalar2=8.0,
                                      op0=Alu.mult, op1=Alu.mult)
                ee = sp.tile([128, HL], f32, name="ee")
                act.activation(ee[:], alf[:], Act.Exp)
                kd = sp.tile([128, HL], f32, name="kd")
                vec.tensor_scalar(out=kd[:], in0=ee[:], scalar1=-1.0, scalar2=1.0,
                                  op0=Alu.mult, op1=Alu.add)
                pc = psB.tile([128, 4], f32, name="pc")
                ten.matmul(pc[:, 0:2], triu_sb[:], alf[:], start=True, stop=True)
                ten.matmul(pc[:, 2:4], slow_sb[:], alf[:], start=True, stop=True)
                expc = sp.tile([128, HL], f32, name="expc")
                act.activation(expc[:], pc[:, 0:2], Act.Exp)
                expnc = sp.tile([128, HL], f32, name="expnc")
                act.activation(expnc[:], pc[:, 0:2], Act.Exp, scale=-1.0)
                expcr = sp.tile([128, HL], f32, name="expcr")
                act.activation(expcr[:], pc[:, 2:4], Act.Exp)
                ksc = sp.tile([128, HL], f32, name="ksc")
                vec.tensor_mul(ksc[:], kd[:], expnc[:])
                dec = sp.tile([128, HL], f32, name="dec")
                vec.tensor_mul(dec[:], expc[:], expcr[:])

                cq = trig_sb["cosq"][:, t, :]
                sq = trig_sb["sinq"][:, t, :]
                ck = trig_sb["cosk"][:, t, :]
                sk = trig_sb["sink"][:, t, :]

                for h in range(HL):
                    qs = slice(h * DH, (h + 1) * DH)
                    ks = slice(256 + h * DH, 256 + (h + 1) * DH)
                    # q~ = xpos(q) * sc * exp(c)   (q read straight from PSUM)
                    qsc = bp.tile([128, 128], f32, name="qsc")
                    vec.tensor_scalar_mul(qsc[:], pqv[:, qs], expc[:, h:h + 1])
                    qrot = bp.tile([128, 128], f32, name="qrot")
                    vec.tensor_mul(qrot[:, 0:64], qsc[:, 0:64], cq)
                    vec.tensor_mul(qrot[:, 64:128], qsc[:, 64:128], cq)
                    qcr = bp.tile([128, 128], f32, name="qcr")
                    vec.tensor_mul(qcr[:, 0:64], qsc[:, 64:128], sq)
                    vec.tensor_mul(qcr[:, 64:128], qsc[:, 0:64], sq)
                    vec.tensor_sub(qrot[:, 0:64], qrot[:, 0:64], qcr[:, 0:64])
                    vec.tensor_add(qrot[:, 64:128], qrot[:, 64:128], qcr[:, 64:128])
                    # k~ = xpos_inv(k) * (1-e^a) * exp(-c)
                    ksl = bp.tile([128, 128], f32, name="ksl")
                    vec.tensor_scalar_mul(ksl[:], pqv[:, ks], ksc[:, h:h + 1])
                    krot = bp.tile([128, 128], f32, name="krot")
                    vec.tensor_mul(krot[:, 0:64], ksl[:, 0:64], ck)
                    vec.tensor_mul(krot[:, 64:128], ksl[:, 64:128], ck)
                    kcr = bp.tile([128, 128], f32, name="kcr")
                    vec.tensor_mul(kcr[:, 0:64], ksl[:, 64:128], sk)
                    vec.tensor_mul(kcr[:, 64:128], ksl[:, 0:64], sk)
                    vec.tensor_sub(krot[:, 0:64], krot[:, 0:64], kcr[:, 0:64])
                    vec.tensor_add(krot[:, 64:128], krot[:, 64:128], kcr[:, 64:128])

                    # transposes to [d, s]
                    pqkT = psB.tile([128, 2, 128], f32, name="pqkT")
                    ten.transpose(pqkT[:, 0, :], qrot[:], ident_sb[:])
                    ten.transpose(pqkT[:, 1, :], krot[:], ident_sb[:])
                    qT = bp.tile([128, 128], f32, name="qT")
                    vec.tensor_copy(qT[:], pqkT[:, 0, :])
                    kT = bp.tile([128, 128], f32, name="kT")
                    vec.tensor_copy(kT[:], pqkT[:, 1, :])

                    # pmix bank: [0]=attnT, [1]=gn-transpose, [2]=k~^T v
                    pmix = psB.tile([128, 3, 128], f32, name="pmix")
                    ten.matmul(pmix[:, 0, :], kT[:], qT[:], start=True, stop=True)
                    am = bp.tile([128, 128], f32, name="am")
                    vec.tensor_mul(am[:], pmix[:, 0, :], triu_sb[:])

                    # ret[i, e] = attn @ v + q~ @ S
                    pr = psB.tile([128, 128], f32, name="pr")
                    ten.matmul(pr[:], am[:], vts[t][:, qs], start=True, stop=False)
                    ten.matmul(pr[:], qT[:], S_sb[:, h, :], start=False, stop=True)

                    # state update: S = dec * (S + k~^T v)
                    ten.matmul(pmix[:, 2, :], krot[:], vts[t][:, qs],
                               start=True, stop=True)
                    Stmp = bp.tile([128, 128], f32, name="Stmp")
                    vec.tensor_add(Stmp[:], S_sb[:, h, :], pmix[:, 2, :])
                    vec.tensor_scalar_mul(S_sb[:, h, :], Stmp[:], dec[:, h:h + 1])

                    # groupnorm stats now; the sqrt + normalize + transpose
                    # happen in one batched pass after the loop (avoids
                    # per-block ACT table-set switching between Exp and Sqrt)
                    bh = t * HL + h
                    st6 = bp.tile([128, 6], f32, name="st6")
                    vec.bn_stats(st6[:], pr[:])
                    vec.bn_aggr(mvall[:, 2 * bh:2 * bh + 2], st6[:])
                    act.copy(rwall[:, bh, :], pr[:])

            # batched groupnorm epilogue: one Sqrt table load for all blocks
            for bh in range(T * HL):
                act.activation(rsall[:, bh:bh + 1], mvall[:, 2 * bh + 1:2 * bh + 2],
                               Act.Sqrt, bias=eps_sb[:, 0:1])
            vec.reciprocal(rsall[:], rsall[:])
            for t in range(T):
                for h in range(HL):
                    bh = t * HL + h
                    xn = bp.tile([128, 128], f32, name="xn")
                    vec.tensor_scalar(out=xn[:], in0=rwall[:, bh, :],
                                      scalar1=mvall[:, 2 * bh:2 * bh + 1],
                                      scalar2=rsall[:, bh:bh + 1],
                                      op0=Alu.subtract, op1=Alu.mult)
                    pxt = psB.tile([128, 2, 128], f32, name="pqkT")
                    ten.transpose(pxt[:, 0, :], xn[:], ident_sb[:])
                    vec.tensor_scalar(out=retn[h][:, t * 128:(t + 1) * 128],
                                      in0=pxt[:, 0, :], scalar1=gnw_sb[:, h:h + 1],
                                      scalar2=gnb_sb[:, h:h + 1], op0=Alu.mult,
                                      op1=Alu.add)

            # state out
            for h in range(HL):
                nc.sync.dma_start(out=io["new_state"].ap()[h], in_=S_sb[:, h, :])

            # pack bounce: bounce[j*256 + h*128 + e, sw] = retn[h][e, j*256+sw]
            for j in range(NCORES):
                for h in range(HL):
                    nc.sync.dma_start(
                        out=bounce[j * CL + h * DH: j * CL + (h + 1) * DH, :],
                        in_=retn[h][:, j * CL:(j + 1) * CL])

        nc.gpsimd.collective_compute(
            "AllToAll", Alu.bypass, replica_groups=RG,
            ins=[bounce.opt()], outs=[recv.opt()],
        )

        # ---- phases C (gate) + D (out), sequence-parallel, bf16 ----
        with (
            tc.tile_pool(name="rtp", bufs=1) as rtp,
            tc.tile_pool(name="wp", bufs=8) as wp,
            tc.tile_pool(name="cp", bufs=2) as cp,
            tc.tile_pool(name="psD", bufs=1, space="PSUM") as psD,
        ):
            rtr = [rtp.tile([128, CL], bf16, name=f"rtr{ci}") for ci in range(T)]
            rt32 = [rtp.tile([128, CL], f32, name=f"rt32_{ci}") for ci in range(T)]
            zr = [rtp.tile([128, CL], bf16, name=f"zr{ci}") for ci in range(T)]
            for ci in range(T):
                nc.gpsimd.dma_start(out=rtr[ci][:], in_=recv[ci * 128:(ci + 1) * 128, :])
                nc.sync.dma_start(out=rt32[ci][:], in_=recv[ci * 128:(ci + 1) * 128, :])

            wg_ap = io["wg"].ap()
            wo_ap = io["wo"].ap()
            for og in range(2):
                pg = [psD.tile([128, CL], f32, name=f"pg{oi}") for oi in range(8)]
                for ci in range(T):
                    wgt = wp.tile([128, 1024], bf16, name="wgt")
                    nc.sync.dma_start(
                        out=wgt, in_=wg_ap[ci * 128:(ci + 1) * 128,
                                          og * 1024:(og + 1) * 1024])
                    for oi in range(8):
                        ten.matmul(pg[oi][:], wgt[:, oi * 128:(oi + 1) * 128],
                                   rtr[ci][:], start=(ci == 0), stop=(ci == T - 1))
                for oi in range(8):
                    o = og * 8 + oi
                    sil = cp.tile([128, CL], f32, name="sil")
                    act.activation(sil[:], pg[oi][:], Act.Silu)
                    z32 = cp.tile([128, CL], f32, name="z32")
                    vec.tensor_mul(z32[:], sil[:], rt32[o][:])
                    nc.gpsimd.dma_start(out=zr[o][:], in_=z32[:])

            for og in range(2):
                po = [psD.tile([128, CL], f32, name=f"pg{oi}") for oi in range(8)]
                for ci in range(T):
                    wot = wp.tile([128, 1024], bf16, name="wot")
                    nc.sync.dma_start(
                        out=wot, in_=wo_ap[ci * 128:(ci + 1) * 128,
                                           og * 1024:(og + 1) * 1024])
                    for oi in range(8):
                        ten.matmul(po[oi][:], wot[:, oi * 128:(oi + 1) * 128],
                                   zr[ci][:], start=(ci == 0), stop=(ci == T - 1))
                for oi in range(8):
                    o = og * 8 + oi
                    ot = cp.tile([128, CL], f32, name="ot")
                    act.copy(ot[:], po[oi][:])
                    nc.sync.dma_start(out=io["outT"].ap()[o * 128:(o + 1) * 128, :],
                                      in_=ot[:])


def _build():
    nc = bacc.Bacc("TRN2", target_bir_lowering=False, debug=False,
                   num_devices=NCORES)
    io = {}
    io["xT"] = nc.dram_tensor("xT", [D, S], f32r, kind="ExternalInput")
    io["wqkv"] = nc.dram_tensor("wqkv", [D, 770], f32r, kind="ExternalInput")
    io["wg"] = nc.dram_tensor("wg", [D, D], bf16, kind="ExternalInput")
    io["wo"] = nc.dram_tensor("wo", [D, D], bf16, kind="ExternalInput")
    io["state"] = nc.dram_tensor("state", [HL, DH, DH], f32, kind="ExternalInput")
    io["balbc"] = nc.dram_tensor("balbc", [128, HL], f32, kind="ExternalInput")
    io["abbc"] = nc.dram_tensor("abbc", [128, HL], f32, kind="ExternalInput")
    io["gnw"] = nc.dram_tensor("gnw", [DH, HL], f32, kind="ExternalInput")
    io["gnb"] = nc.dram_tensor("gnb", [DH, HL], f32, kind="ExternalInput")
    for k in ("cosq", "sinq", "cosk", "sink"):
        io[k] = nc.dram_tensor(k, [S, 64], f32, kind="ExternalInput")
    io["triu"] = nc.dram_tensor("triu", [128, 128], f32, kind="ExternalInput")
    io["slow"] = nc.dram_tensor("slow", [128, 128], f32, kind="ExternalInput")
    io["ident"] = nc.dram_tensor("ident", [128, 128], f32, kind="ExternalInput")
    io["outT"] = nc.dram_tensor("outT", [D, CL], f32, kind="ExternalOutput")
    io["new_state"] = nc.dram_tensor("new_state", [HL, DH, DH], f32,
                                     kind="ExternalOutput")

    with tile.TileContext(nc) as tc:
        _emit(nc, tc, io)
    nc.compile()
    return nc


def _prep_inputs(x, state, W_qkv, W_alpha, b_alpha, alpha_base, gn_weight,
                 gn_bias, W_out, W_gate, offset):
    asf = lambda a: np.ascontiguousarray(np.asarray(a, dtype=np.float32))
    x = asf(x); state = asf(state); W_qkv = asf(W_qkv); W_alpha = asf(W_alpha)
    b_alpha = asf(b_alpha); alpha_base = asf(alpha_base)
    gn_weight = asf(gn_weight); gn_bias = asf(gn_bias)
    W_out = asf(W_out); W_gate = asf(W_gate)
    off = float(np.asarray(offset))

    xT = np.ascontiguousarray(x.T)
    wgT = np.ascontiguousarray(W_gate.T.astype(ml_dtypes.bfloat16))
    woT = np.ascontiguousarray(W_out.T.astype(ml_dtypes.bfloat16))

    half = 64
    freq = (1.0 / (10000.0 ** (np.arange(half, dtype=np.float32) / half)))
    pos = np.arange(S, dtype=np.float32) + np.float32(off)
    ang = pos[:, None] * freq[None, :]
    zeta = ((np.arange(half, dtype=np.float32) * 2.0 + 0.4 * DH) / (1.4 * DH))
    sc = zeta[None, :] ** (pos[:, None] / 512.0)
    cosq = asf(np.cos(ang) * sc)
    sinq = asf(np.sin(ang) * sc)
    cosk = asf(np.cos(ang) / sc)
    sink = asf(np.sin(ang) / sc)
    triu = asf(np.triu(np.ones((128, 128), dtype=np.float32)))
    slow = asf(np.tril(np.ones((128, 128), dtype=np.float32), -1))
    ident = asf(np.eye(128, dtype=np.float32))

    in_maps = []
    for r in range(NCORES):
        h0 = HL * r
        wq = W_qkv[h0 * DH:(h0 + HL) * DH]
        wk = W_qkv[D + h0 * DH: D + (h0 + HL) * DH]
        wv = W_qkv[2 * D + h0 * DH: 2 * D + (h0 + HL) * DH]
        wa = W_alpha[h0:h0 + HL]
        wqkvT = np.ascontiguousarray(np.concatenate([wq, wk, wv, wa], axis=0).T)
        in_maps.append({
            "xT": xT,
            "wqkv": wqkvT,
            "wg": wgT,
            "wo": woT,
            "state": np.ascontiguousarray(state[h0:h0 + HL]),
            "balbc": np.ascontiguousarray(np.tile(b_alpha[h0:h0 + HL][None, :], (128, 1))),
            "abbc": np.ascontiguousarray(np.tile(alpha_base[h0:h0 + HL][None, :], (128, 1))),
            "gnw": np.ascontiguousarray(gn_weight[h0 * DH:(h0 + HL) * DH].reshape(HL, DH).T),
            "gnb": np.ascontiguousarray(gn_bias[h0 * DH:(h0 + HL) * DH].reshape(HL, DH).T),
            "cosq": cosq, "sinq": sinq, "cosk": cosk, "sink": sink,
            "triu": triu, "slow": slow, "ident": ident,
        })
    return in_maps


def kernel(x, state, W_qkv, W_alpha, b_alpha, alpha_base, gn_weight, gn_bias,
           W_out, W_gate, offset, _trace=False):
    global _compiled
    if _compiled is None:
        _compiled = _build()
    nc = _compiled
    in_maps = _prep_inputs(x, state, W_qkv, W_alpha, b_alpha, alpha_base,
                           gn_weight, gn_bias, W_out, W_gate, offset)
    res = bass_utils.run_bass_kernel_spmd(nc, in_maps,
                                          core_ids=list(range(NCORES)),
                                          trace=_trace)
    kernel.last_results = res
    out = np.concatenate([res.results[r]["outT"] for r in range(NCORES)],
                         axis=1).T.copy()
    new_state = np.concatenate([res.results[r]["new_state"]
                                for r in range(NCORES)], axis=0)
    return out, new_state
